# revision 30
# baseline (speedup 1.0000x reference)
"""Trainium2 Bass kernel for nn_PositionalEncoding (gnn_message_passing).

Self-contained: takes FULL inputs, shards across 8 NeuronCores internally,
runs one SPMD Bass program, reassembles the full output on the host.

Math (per reference):
  deg  = relu(deg_emb[tree_degree] @ W1 + b1)
  x    = (x_clique + deg) @ Wm + mb
  tpe  = nan0(tree_lpe) @ tlw + tlb
  pe   = nan0(graph_lpe) @ lpw + lpb
  pec  = segment_mean(pe[row], col)        (0 where count==0)
  out  = x + concat([pec, tpe], -1)

v2 design notes (vs the fp32 baseline):
  - the degree path is a 100-row table lookup; it is folded into x on the
    host (xp = x_clique + T[tree_degree], T = relu(deg_emb@W1+b1)), so the
    device only computes xp @ Wm.
  - all device streams are bf16 (PE runs 1 cyc/col vs 4 for fp32; DMA bytes
    halve).  Matmul accumulation stays fp32 in PSUM.
  - per super-group of 8 clique groups there are exactly 3 DMAs: one packed
    input stream ([x: 512][gather blocks: sum_t 32*k_t] per group), one
    [32, 4096] tree-lpe block, one [128, 4096] output store.  Per-DMA
    overhead on trn2 is ~0.6us serialized, so DMA count matters.
  - per group of 512 cliques: one 512-col wm matmul (start=True owns the
    PSUM bank -> no memset), one 512-col tpe matmul into rows 64:128, a
    bf16 DVE strided reduce of the gathered edge rows ((f s) layout,
    innermost stride 1; uniform-k groups fuse all 4 tiles into one
    instruction), 4 PE transposes into a bf16 PSUM tile, one DVE copy to
    SBUF, 4 lpe matmuls with per-class (lpw * 1/k) weights, and one ACT
    bias-add copy PSUM -> bf16 output tile.
"""

import math

import numpy as np

N_CORES = 8
HID = 128
PE = 32
P = 128          # partitions / clique-tile size
GROUP = 4        # clique tiles per group (4 * 128 = 512 = one PSUM bank)
GW = GROUP * P   # 512
SG = 8           # groups per super-group (one input DMA + one output DMA)

_COMPILE_CACHE: dict = {}


def _bf16():
    from concourse import mybir
    return mybir.dt.np(mybir.dt.bfloat16)


# --------------------------------------------------------------------------
# planning (shared across cores -> one SPMD program)
# --------------------------------------------------------------------------

def _plan(cnts_list, kmax):
    """Build the uniform class/tile/group/stream structure from per-core
    per-clique edge counts."""
    K = kmax
    ncls = np.zeros((len(cnts_list), K + 1), np.int64)
    for c, cnt in enumerate(cnts_list):
        b = np.bincount(cnt, minlength=K + 1)
        ncls[c, : len(b)] = b[: K + 1]
    # tiles per class: max over cores, so the program is core-independent
    n = [int(max((ncls[c, k] + P - 1) // P for c in range(len(cnts_list))))
         for k in range(K + 1)]
    n[0] = max(n[0], 1)
    n[0] += (-n[0]) % GROUP  # class-0 section group-aligned
    rest = sum(n[1:])
    if rest % GROUP:
        klast = max(k for k in range(1, K + 1) if n[k] > 0)
        n[klast] += (-rest) % GROUP

    classes = [k for k in range(K + 1) if n[k] > 0]  # 0 first, then ascending
    tiles = []           # global tile list -> class k
    class_tile0 = {}     # class -> first global tile index
    for k in classes:
        class_tile0[k] = len(tiles)
        tiles += [k] * n[k]
    n_t = len(tiles)
    assert n_t % GROUP == 0

    # Stream layout.  A tile of class k owns ceil(k/4) blocks of 128 cols;
    # block L (lane L) holds slots 4L..4L+3 spread across partition strips
    # (partition = 32*j + feat).  Full blocks (4 strips) live in the main
    # [128, *] stream behind the x columns; partial blocks (w = 1..3
    # strips) live in narrow aux streams [32w, *] so no zero-padding is
    # ever transferred.  The device segment-sums each block run with
    # strip-replicated lpw weights in one matmul (slot sum + projection).
    groups = []
    col = 0
    acols = {1: 0, 2: 0, 3: 0}
    for gi in range(n_t // GROUP):
        ks = tuple(tiles[gi * GROUP:(gi + 1) * GROUP])
        class0 = (ks[0] == 0)
        assert class0 == (ks[-1] == 0), "mixed class-0 group"
        x0 = col
        col += GW
        qs = [(k + 3) // 4 for k in ks]
        tlc = {}      # (tile_in_group, lane) -> (width, col in its stream)
        segs = []     # merged matmul runs: w/col0/out0/ncols
        for L in range(max(qs) if qs else 0):
            for t in range(GROUP):
                if qs[t] <= L:
                    continue
                w = min(4, ks[t] - 4 * L)
                if w == 4:
                    c = col
                    col += P
                else:
                    c = acols[w]
                    acols[w] += P
                tlc[(t, L)] = (w, c)
                if (segs and segs[-1]["w"] == w
                        and segs[-1]["t_end"] == t
                        and segs[-1]["col0"] + segs[-1]["ncols"] == c):
                    segs[-1]["ncols"] += P
                    segs[-1]["t_end"] = t + 1
                else:
                    segs.append(dict(w=w, col0=c, out0=t * P, ncols=P,
                                     t_end=t + 1))
        groups.append(dict(off=gi * GW, ks=ks, class0=class0, x0=x0,
                           segs=segs, tlc=tlc, end=col,
                           aend=dict(acols)))
    s_cols = col
    a_cols = dict(acols)

    sgs = []
    astart = {1: 0, 2: 0, 3: 0}
    for s in range(0, len(groups), SG):
        gg = groups[s:s + SG]
        ac0 = dict(astart)
        acn = {w: gg[-1]["aend"][w] - ac0[w] for w in (1, 2, 3)}
        astart = dict(gg[-1]["aend"])
        sgs.append(dict(c0=gg[0]["x0"], cols=gg[-1]["end"] - gg[0]["x0"],
                        out0=gg[0]["off"], groups=gg, ac0=ac0, acn=acn))
    max_sg_cols = max(sg["cols"] for sg in sgs)
    max_a_cols = {w: max((sg["acn"][w] for sg in sgs), default=0)
                  for w in (1, 2, 3)}

    return dict(n=n, classes=classes, class_tile0=class_tile0, tiles=tiles,
                n_t=n_t, np_=n_t * P, groups=groups, sgs=sgs,
                s_cols=s_cols, max_sg_cols=max_sg_cols,
                a_cols=a_cols, max_a_cols=max_a_cols)


def _perm_arrays(plan, cnt):
    """Permutation position->local clique id for one core."""
    NP = plan["np_"]
    perm = np.full(NP, -1, np.int64)
    for k in plan["classes"]:
        ids = np.flatnonzero(cnt == k)
        base = plan["class_tile0"][k] * P
        perm[base:base + len(ids)] = ids
    realpos = np.flatnonzero(perm >= 0)
    realids = perm[realpos]
    return perm, realpos, realids


def _core_stream(plan, xp16, perm, crow_s, starts, n_atoms, glpe_pad):
    """Per-core packed input stream [128, s_cols] bf16.

    Gather blocks are slot-spread: block (tile, lane L) is [128, 128] with
    partition 32*j + f = (slot 4L+j, feat f), col = clique-in-tile, values
    pre-scaled by 1/k so the device matmul computes the segment mean.
    """
    BF16 = _bf16()
    NP = plan["np_"]
    stream = np.zeros((P, plan["s_cols"]), BF16)
    aux = {w: np.zeros((PE * w, max(plan["a_cols"][w], 1)), BF16)
           for w in (1, 2, 3)}

    xT = np.zeros((P, NP), BF16)
    realpos = np.flatnonzero(perm >= 0)
    xT[:, realpos] = xp16.T

    for grp in plan["groups"]:
        g0 = grp["off"]
        stream[:, grp["x0"]:grp["x0"] + GW] = xT[:, g0:g0 + GW]

    # gather blocks, per class (tiles of one class are contiguous)
    for k in plan["classes"]:
        if k == 0:
            continue
        q = (k + 3) // 4
        t0 = plan["class_tile0"][k]
        nk = plan["n"][k]
        idmat = perm[t0 * P:(t0 + nk) * P].reshape(nk, P)
        st = np.where(idmat >= 0, starts[idmat.clip(0)], 0)
        base = st[..., None] + np.arange(k)[None, None, :]   # [nk, P, k]
        vals = crow_s[base.clip(0, max(len(crow_s) - 1, 0))]
        vals[idmat < 0] = n_atoms
        rows = (glpe_pad[vals] * np.float32(1.0 / k)).astype(BF16)
        for i in range(nk):
            t = t0 + i
            grp = plan["groups"][t // GROUP]
            tt = t % GROUP
            X = rows[i]                                       # [P, k, 32]
            for L in range(q):
                w, c0b = grp["tlc"][(tt, L)]
                blk = X[:, 4 * L:4 * L + w, :].transpose(1, 2, 0)
                dst = stream if w == 4 else aux[w]
                dst[:, c0b:c0b + P] = blk.reshape(PE * w, P)
    return stream, aux


# --------------------------------------------------------------------------
# Bass program
# --------------------------------------------------------------------------

def _build_bass(plan, repeat=None, mode="full"):
    """mode: "full" (default) | "dma" (loads/stores only) | "compute"
    (no big DMAs; engines read a memset dummy tile).  The non-full modes
    exist only for on-hardware bottleneck attribution."""
    import concourse.bass as bass
    import concourse.bacc as bacc
    import concourse.mybir as mybir
    import concourse.tile as tile
    from concourse.masks import make_identity

    f32 = mybir.dt.float32
    bf16 = mybir.dt.bfloat16
    NP = plan["np_"]
    # consts layout: [wm: 128][tlw: 64][strip-spread lpw variants 1..4: 64
    # cols each; variant L replicates lpw on partition strips j < L]
    C_WM, C_TLW, C_SP = 0, HID, HID + 64
    c_cols = C_SP + 4 * 64

    nc = bacc.Bacc(None)
    d_stream = nc.declare_dram_parameter("stream", [P, plan["s_cols"]], bf16,
                                         isOutput=False)
    d_aux = {w: nc.declare_dram_parameter(
                 f"aux{w}", [PE * w, max(plan["a_cols"][w], 1)], bf16,
                 isOutput=False)
             for w in (1, 2, 3)}
    d_tl = nc.declare_dram_parameter("tlT", [PE, NP], bf16, isOutput=False)
    d_consts = nc.declare_dram_parameter("consts", [P, c_cols], bf16,
                                         isOutput=False)
    d_bias = nc.declare_dram_parameter("bias", [HID, 2], f32, isOutput=False)
    d_out = nc.declare_dram_parameter("outT", [P, NP], bf16, isOutput=True)

    with tile.TileContext(nc) as tc:
        with (
            tc.tile_pool(name="const", bufs=1) as cp,
            tc.tile_pool(name="st", bufs=3) as spool,
            tc.tile_pool(name="a1", bufs=3) as a1pool,
            tc.tile_pool(name="a2", bufs=3) as a2pool,
            tc.tile_pool(name="a3", bufs=3) as a3pool,
            tc.tile_pool(name="tl", bufs=3) as tlpool,
            tc.tile_pool(name="ot", bufs=3) as opool,
            tc.tile_pool(name="psF", bufs=8, space="PSUM") as psF,
        ):
            apools = {1: a1pool, 2: a2pool, 3: a3pool}
            # ---------------- constants ----------------
            cw = cp.tile([P, c_cols], bf16, tag="cw")
            nc.sync.dma_start(out=cw[:], in_=d_consts[:, :])
            bias_sb = cp.tile([HID, 2], f32, tag="bias")
            nc.sync.dma_start(out=bias_sb[:], in_=d_bias[:, :])
            if mode == "compute":
                fake_st = cp.tile([P, 2048], bf16, tag="fst")
                nc.vector.memset(fake_st[:], 0.25)
                fake_tl = cp.tile([PE, GW], bf16, tag="ftl")
                nc.vector.memset(fake_tl[:], 0.25)

            # ---------------- main loop ----------------
            import contextlib
            rep_ctx = (tc.For_i(0, repeat, 1) if repeat
                       else contextlib.nullcontext())
            rep_ctx.__enter__()

            for sg in plan["sgs"]:
                c0 = sg["c0"]
                ng = len(sg["groups"])
                if mode != "compute":
                    st = spool.tile([P, plan["max_sg_cols"]], bf16,
                                    tag="st")
                    nc.sync.dma_start(out=st[:, :sg["cols"]],
                                      in_=d_stream[:, c0:c0 + sg["cols"]])
                    ax = {}
                    for w in (1, 2, 3):
                        nw = sg["acn"][w]
                        if plan["max_a_cols"][w] == 0:
                            continue
                        at = apools[w].tile([PE * w, plan["max_a_cols"][w]],
                                            bf16, tag=f"ax{w}")
                        ax[w] = at
                        if nw:
                            a0 = sg["ac0"][w]
                            nc.sync.dma_start(out=at[:, :nw],
                                              in_=d_aux[w][:, a0:a0 + nw])
                    tl = tlpool.tile([PE, SG * GW], bf16, tag="tl")
                    nc.sync.dma_start(
                        out=tl[:, :ng * GW],
                        in_=d_tl[:, sg["out0"]:sg["out0"] + ng * GW])
                ot = opool.tile([P, SG * GW], bf16, tag="ot")
                if mode == "dma":
                    nc.vector.memset(ot[:, 0:1], 0.0)

                # whole-sg phases (8 PSUM banks = 8 groups in flight):
                # all wm matmuls, then all tpe, then all lpe segment-sums,
                # then the ACT bias-add output copies.  One weight load per
                # phase run; PE streams with no cross-engine dependencies.
                glist = list(enumerate(sg["groups"])) if mode != "dma" else []
                fins = {}
                for gl, grp in glist:
                    fin = psF.tile([P, GW], f32)
                    fins[gl] = fin
                for gl, grp in glist:
                    if mode == "compute":
                        xs_ap = fake_st[:, 0:GW]
                    else:
                        xs_ap = st[:, grp["x0"] - c0:grp["x0"] - c0 + GW]
                    nc.tensor.matmul(fins[gl][:, :],
                                     lhsT=cw[:, C_WM:C_WM + HID],
                                     rhs=xs_ap, start=True, stop=False,
                                     skip_group_check=True)
                for gl, grp in glist:
                    if mode == "compute":
                        tl_ap = fake_tl[:, 0:GW]
                    else:
                        tl_ap = tl[:, gl * GW:(gl + 1) * GW]
                    nc.tensor.matmul(fins[gl][64:128, :],
                                     lhsT=cw[0:PE, C_TLW:C_TLW + 64],
                                     rhs=tl_ap,
                                     start=False, stop=grp["class0"],
                                     skip_group_check=True)
                # lpe: merged slot-spread segment matmuls (sum over edge
                # slots and project in one op, accumulating into fin[0:64]);
                # width-w segments read the [32w, *] aux stream with the
                # strip-replicated lpw sliced to w strips
                for gl, grp in glist:
                    for i, seg in enumerate(grp["segs"]):
                        w = seg["w"]
                        co = C_SP + (w - 1) * 64
                        if mode == "compute":
                            rhs = (fake_st[0:PE * w, 0:seg["ncols"]])
                        elif w == 4:
                            rhs = st[:, seg["col0"] - c0:
                                     seg["col0"] - c0 + seg["ncols"]]
                        else:
                            a0 = sg["ac0"][w]
                            rhs = ax[w][:, seg["col0"] - a0:
                                        seg["col0"] - a0 + seg["ncols"]]
                        nc.tensor.matmul(
                            fins[gl][0:64,
                                     seg["out0"]:seg["out0"] + seg["ncols"]],
                            lhsT=cw[0:PE * w, co:co + 64], rhs=rhs,
                            start=False, stop=(i == len(grp["segs"]) - 1),
                            skip_group_check=True)
                # bias-add copies PSUM -> bf16 output tile (ACT/DVE split)
                for gl, grp in glist:
                    bcol = 0 if grp["class0"] else 1
                    if gl % 2 == 0:
                        nc.scalar.add(ot[:, gl * GW:(gl + 1) * GW],
                                      fins[gl][:, :],
                                      bias_sb[:, bcol:bcol + 1])
                    else:
                        nc.vector.tensor_scalar(
                            out=ot[:, gl * GW:(gl + 1) * GW],
                            in0=fins[gl][:, :],
                            scalar1=bias_sb[:, bcol:bcol + 1], scalar2=None,
                            op0=mybir.AluOpType.add)

                # issue the store from the ACT queue: SP's in-order SEQ
                # would otherwise park on this DMA's wait and stall the
                # next super-group's input DMA dispatch (no overlap).
                nc.scalar.dma_start(
                    out=d_out[:, sg["out0"]:sg["out0"] + ng * GW],
                    in_=ot[:, :ng * GW])

            rep_ctx.__exit__(None, None, None)

    nc.compile()
    return nc


# --------------------------------------------------------------------------
# SPMD execution via PJRT (axon)
# --------------------------------------------------------------------------

def _run_spmd(nc, in_maps, bench=None):
    import jax
    import numpy as np
    from jax.sharding import Mesh, PartitionSpec
    from jax.experimental.shard_map import shard_map
    from concourse import bass2jax, mybir
    from concourse.bass2jax import _bass_exec_p, partition_id_tensor

    bass2jax.install_neuronx_cc_hook()
    n_cores = len(in_maps)
    partition_name = nc.partition_id_tensor.name if nc.partition_id_tensor else None
    in_names, out_names, out_avals, zero_outs = [], [], [], []
    for alloc in nc.m.functions[0].allocations:
        if not isinstance(alloc, mybir.MemoryLocationSet):
            continue
        name = alloc.memorylocations[0].name
        if alloc.kind == "ExternalInput":
            if name != partition_name:
                in_names.append(name)
        elif alloc.kind == "ExternalOutput":
            out_names.append(name)
            shape = tuple(alloc.tensor_shape)
            dtype = mybir.dt.np(alloc.dtype)
            out_avals.append(jax.core.ShapedArray(shape, dtype))
            zero_outs.append(np.zeros(shape, dtype))
    n_params = len(in_names)
    n_outs = len(out_avals)
    in_names.extend(out_names)
    if partition_name is not None:
        in_names.append(partition_name)

    def _body(*args):
        operands = list(args)
        if partition_name is not None:
            operands.append(partition_id_tensor())
        return tuple(_bass_exec_p.bind(
            *operands, out_avals=tuple(out_avals), in_names=tuple(in_names),
            out_names=tuple(out_names), lowering_input_output_aliases=(),
            sim_require_finite=True, sim_require_nnan=True, nc=nc))

    devices = jax.devices()[:n_cores]
    mesh = Mesh(np.asarray(devices), ("core",))
    in_specs = (PartitionSpec("core"),) * (n_params + n_outs)
    out_specs = (PartitionSpec("core"),) * len(out_names)
    sharded = jax.jit(shard_map(_body, mesh=mesh, in_specs=in_specs,
                                out_specs=out_specs, check_rep=False),
                      keep_unused=True)
    concat_in = [np.concatenate([np.asarray(m[in_names[i]]) for m in in_maps], axis=0)
                 for i in range(n_params)]
    concat_zeros = [np.zeros((n_cores * z.shape[0], *z.shape[1:]), z.dtype)
                    for z in zero_outs]
    sharding = jax.sharding.NamedSharding(mesh, PartitionSpec("core"))
    dev_in = [jax.device_put(a, sharding) for a in concat_in + concat_zeros]
    out_arrs = jax.block_until_ready(sharded(*dev_in))

    if bench is not None:
        import time
        iters = int(bench.get("iters", 10))
        times = []
        for _ in range(iters):
            t0 = time.perf_counter()
            jax.block_until_ready(sharded(*dev_in))
            times.append(time.perf_counter() - t0)
        bench["times"] = times
        bench["min_wall_ns"] = int(min(times) * 1e9)

    return [{name: np.asarray(out_arrs[i]).reshape(n_cores, *out_avals[i].shape)[c]
             for i, name in enumerate(out_names)} for c in range(n_cores)]


# --------------------------------------------------------------------------
# entry point
# --------------------------------------------------------------------------

def kernel(x_clique, tree_lpe, graph_lpe, tree_degree, row, col,
           deg_emb, deg_lin_w, deg_lin_b, deg_merge_w, deg_merge_b,
           tree_lpe_w, tree_lpe_b, lpe_w, lpe_b, _bench=None):
    BF16 = _bf16()

    x_clique = np.asarray(x_clique, np.float32)
    tree_lpe = np.asarray(tree_lpe, np.float32)
    graph_lpe = np.asarray(graph_lpe, np.float32)
    tree_degree = np.asarray(tree_degree).astype(np.int64)
    row = np.asarray(row).astype(np.int64)
    col = np.asarray(col).astype(np.int64)
    deg_emb = np.asarray(deg_emb, np.float32)
    deg_lin_w = np.asarray(deg_lin_w, np.float32)
    deg_lin_b = np.asarray(deg_lin_b, np.float32)
    deg_merge_w = np.asarray(deg_merge_w, np.float32)
    deg_merge_b = np.asarray(deg_merge_b, np.float32)
    tree_lpe_w = np.asarray(tree_lpe_w, np.float32)
    tree_lpe_b = np.asarray(tree_lpe_b, np.float32)
    lpe_w = np.asarray(lpe_w, np.float32)
    lpe_b = np.asarray(lpe_b, np.float32)

    n_clique = x_clique.shape[0]
    n_atoms = graph_lpe.shape[0]
    assert n_clique % N_CORES == 0
    cpc = n_clique // N_CORES

    # degree table folded on host: T = relu(deg_emb @ W1 + b1)
    degfeat = np.maximum(deg_emb @ deg_lin_w + deg_lin_b, 0.0)

    # ---- host index prep: partition edges by owning core, count per clique
    order = np.argsort(col, kind="stable")
    col_s = col[order]
    row_s = row[order]
    bounds = np.searchsorted(col_s, np.arange(N_CORES + 1) * cpc)

    cnts, crows = [], []
    for c in range(N_CORES):
        lo, hi = bounds[c], bounds[c + 1]
        cc = col_s[lo:hi] - c * cpc
        cnts.append(np.bincount(cc, minlength=cpc).astype(np.int64))
        crows.append(row_s[lo:hi])

    kmax = int(max(int(c.max(initial=0)) for c in cnts))
    plan = _plan(cnts, kmax)

    glpe_pad = np.vstack([np.nan_to_num(graph_lpe, nan=0.0),
                          np.zeros((1, PE), np.float32)])

    # consts: [wm 128][tlw 64][strip-spread lpw variants 1..4]  (bf16)
    c_cols = HID + 64 + 4 * 64
    consts = np.zeros((P, c_cols), BF16)
    consts[:, :HID] = deg_merge_w.astype(BF16)
    consts[0:PE, HID:HID + 64] = tree_lpe_w.astype(BF16)
    lpw16 = lpe_w.astype(BF16)
    for L in range(1, 5):
        co = HID + 64 + (L - 1) * 64
        for j in range(L):
            consts[PE * j:PE * (j + 1), co:co + 64] = lpw16

    bias = np.zeros((HID, 2), np.float32)
    bias[:, 0] = deg_merge_b + np.concatenate([np.zeros(64, np.float32),
                                               tree_lpe_b])
    bias[:, 1] = bias[:, 0] + np.concatenate([lpe_b, np.zeros(64, np.float32)])

    in_maps = []
    unshard = []
    for c in range(N_CORES):
        cnt = cnts[c]
        perm, realpos, realids = _perm_arrays(plan, cnt)
        crow_s = crows[c]
        starts = np.zeros(cpc, np.int64)
        cs = np.cumsum(cnt)
        starts[1:] = cs[:-1]

        x_c = x_clique[c * cpc:(c + 1) * cpc]
        tl_c = tree_lpe[c * cpc:(c + 1) * cpc]
        deg_c = tree_degree[c * cpc:(c + 1) * cpc]

        xp16 = (x_c[realids] + degfeat[deg_c[realids]]).astype(BF16)
        tlT = np.zeros((PE, plan["np_"]), BF16)
        tlT[:, realpos] = np.nan_to_num(tl_c[realids], nan=0.0).astype(BF16).T

        stream, aux = _core_stream(plan, xp16, perm, crow_s, starts, n_atoms,
                                   glpe_pad)
        in_maps.append(dict(stream=stream, aux1=aux[1], aux2=aux[2],
                            aux3=aux[3], tlT=tlT, consts=consts, bias=bias))
        unshard.append((realpos, realids))

    cache_key = (tuple(plan["tiles"]),)
    nc = _COMPILE_CACHE.get(cache_key)
    if nc is None:
        nc = _build_bass(plan)
        _COMPILE_CACHE[cache_key] = nc

    results = _run_spmd(nc, in_maps, bench=_bench)

    # true HW time: run repeat-R variants of the program (device-side loop);
    # the wall-time slope vs R is pure device time, dispatch cancels out.
    if _bench is not None and _bench.get("hw_probe"):
        walls = {}
        for R in _bench["hw_probe"]:
            ncR = _build_bass(plan, repeat=R)
            b2 = {"iters": _bench.get("iters", 8)}
            _run_spmd(ncR, in_maps, bench=b2)
            walls[R] = min(b2["times"])
        rs = sorted(walls)
        _bench["walls"] = walls
        _bench["hw_ns_est"] = int(
            (walls[rs[-1]] - walls[rs[0]]) / (rs[-1] - rs[0]) * 1e9)

    out = np.empty((n_clique, HID), np.float32)
    for c in range(N_CORES):
        realpos, realids = unshard[c]
        outT = results[c]["outT"]  # [128, NP] bf16
        out[c * cpc + realids] = outT.T[realpos].astype(np.float32)
    return out


# revision 31
# speedup vs baseline: 1.4229x; 1.4229x over previous
"""Trainium2 Bass kernel for nn_PositionalEncoding (gnn_message_passing).

Self-contained: takes FULL inputs, shards across 8 NeuronCores internally,
runs one SPMD Bass program, reassembles the full output on the host.

Math (per reference):
  deg  = relu(deg_emb[tree_degree] @ W1 + b1)
  x    = (x_clique + deg) @ Wm + mb
  tpe  = nan0(tree_lpe) @ tlw + tlb
  pe   = nan0(graph_lpe) @ lpw + lpb
  pec  = segment_mean(pe[row], col)        (0 where count==0)
  out  = x + concat([pec, tpe], -1)

v2 design notes (vs the fp32 baseline):
  - the degree path is a 100-row table lookup; it is folded into x on the
    host (xp = x_clique + T[tree_degree], T = relu(deg_emb@W1+b1)), so the
    device only computes xp @ Wm.
  - all device streams are bf16 (PE runs 1 cyc/col vs 4 for fp32; DMA bytes
    halve).  Matmul accumulation stays fp32 in PSUM.
  - per super-group of 8 clique groups there are exactly 3 DMAs: one packed
    input stream ([x: 512][gather blocks: sum_t 32*k_t] per group), one
    [32, 4096] tree-lpe block, one [128, 4096] output store.  Per-DMA
    overhead on trn2 is ~0.6us serialized, so DMA count matters.
  - per group of 512 cliques: one 512-col wm matmul (start=True owns the
    PSUM bank -> no memset), one 512-col tpe matmul into rows 64:128, a
    bf16 DVE strided reduce of the gathered edge rows ((f s) layout,
    innermost stride 1; uniform-k groups fuse all 4 tiles into one
    instruction), 4 PE transposes into a bf16 PSUM tile, one DVE copy to
    SBUF, 4 lpe matmuls with per-class (lpw * 1/k) weights, and one ACT
    bias-add copy PSUM -> bf16 output tile.
"""

import math

import numpy as np

N_CORES = 8
HID = 128
PE = 32
P = 128          # partitions / clique-tile size
GROUP = 4        # clique tiles per group (4 * 128 = 512 = one PSUM bank)
GW = GROUP * P   # 512
SG = 8           # groups per super-group (one input DMA + one output DMA)

_COMPILE_CACHE: dict = {}


def _bf16():
    from concourse import mybir
    return mybir.dt.np(mybir.dt.bfloat16)


# --------------------------------------------------------------------------
# planning (shared across cores -> one SPMD program)
# --------------------------------------------------------------------------

def _plan(cnts_list, kmax):
    """Build the uniform class/tile/group/stream structure from per-core
    per-clique edge counts."""
    K = kmax
    ncls = np.zeros((len(cnts_list), K + 1), np.int64)
    for c, cnt in enumerate(cnts_list):
        b = np.bincount(cnt, minlength=K + 1)
        ncls[c, : len(b)] = b[: K + 1]
    # tiles per class: max over cores, so the program is core-independent
    n = [int(max((ncls[c, k] + P - 1) // P for c in range(len(cnts_list))))
         for k in range(K + 1)]
    n[0] = max(n[0], 1)
    n[0] += (-n[0]) % GROUP  # class-0 section group-aligned
    rest = sum(n[1:])
    if rest % GROUP:
        klast = max(k for k in range(1, K + 1) if n[k] > 0)
        n[klast] += (-rest) % GROUP

    classes = [k for k in range(K + 1) if n[k] > 0]  # 0 first, then ascending
    tiles = []           # global tile list -> class k
    class_tile0 = {}     # class -> first global tile index
    for k in classes:
        class_tile0[k] = len(tiles)
        tiles += [k] * n[k]
    n_t = len(tiles)
    assert n_t % GROUP == 0

    # Stream layout.  A tile of class k owns ceil(k/4) blocks of 128 cols;
    # block L (lane L) holds slots 4L..4L+3 spread across partition strips
    # (partition = 32*j + feat).  Full blocks (4 strips) live in the main
    # [128, *] stream behind the x columns; partial blocks (w = 1..3
    # strips) live in narrow aux streams [32w, *] so no zero-padding is
    # ever transferred.  The device segment-sums each block run with
    # strip-replicated lpw weights in one matmul (slot sum + projection).
    groups = []
    col = 0
    acols = {1: 0, 2: 0, 3: 0}
    for gi in range(n_t // GROUP):
        ks = tuple(tiles[gi * GROUP:(gi + 1) * GROUP])
        class0 = (ks[0] == 0)
        assert class0 == (ks[-1] == 0), "mixed class-0 group"
        x0 = col
        col += GW
        qs = [(k + 3) // 4 for k in ks]
        tlc = {}      # (tile_in_group, lane) -> (width, col in its stream)
        segs = []     # merged matmul runs: w/col0/out0/ncols
        for L in range(max(qs) if qs else 0):
            for t in range(GROUP):
                if qs[t] <= L:
                    continue
                w = min(4, ks[t] - 4 * L)
                if w == 4:
                    c = col
                    col += P
                else:
                    c = acols[w]
                    acols[w] += P
                tlc[(t, L)] = (w, c)
                if (segs and segs[-1]["w"] == w
                        and segs[-1]["t_end"] == t
                        and segs[-1]["col0"] + segs[-1]["ncols"] == c):
                    segs[-1]["ncols"] += P
                    segs[-1]["t_end"] = t + 1
                else:
                    segs.append(dict(w=w, col0=c, out0=t * P, ncols=P,
                                     t_end=t + 1))
        groups.append(dict(off=gi * GW, ks=ks, class0=class0, x0=x0,
                           segs=segs, tlc=tlc, end=col,
                           aend=dict(acols)))
    s_cols = col
    a_cols = dict(acols)

    sgs = []
    astart = {1: 0, 2: 0, 3: 0}
    for s in range(0, len(groups), SG):
        gg = groups[s:s + SG]
        ac0 = dict(astart)
        acn = {w: gg[-1]["aend"][w] - ac0[w] for w in (1, 2, 3)}
        astart = dict(gg[-1]["aend"])
        sgs.append(dict(c0=gg[0]["x0"], cols=gg[-1]["end"] - gg[0]["x0"],
                        out0=gg[0]["off"], groups=gg, ac0=ac0, acn=acn))
    max_sg_cols = max(sg["cols"] for sg in sgs)
    max_a_cols = {w: max((sg["acn"][w] for sg in sgs), default=0)
                  for w in (1, 2, 3)}

    return dict(n=n, classes=classes, class_tile0=class_tile0, tiles=tiles,
                n_t=n_t, np_=n_t * P, groups=groups, sgs=sgs,
                s_cols=s_cols, max_sg_cols=max_sg_cols,
                a_cols=a_cols, max_a_cols=max_a_cols)


def _perm_arrays(plan, cnt):
    """Permutation position->local clique id for one core."""
    NP = plan["np_"]
    perm = np.full(NP, -1, np.int64)
    for k in plan["classes"]:
        ids = np.flatnonzero(cnt == k)
        base = plan["class_tile0"][k] * P
        perm[base:base + len(ids)] = ids
    realpos = np.flatnonzero(perm >= 0)
    realids = perm[realpos]
    return perm, realpos, realids


def _core_stream(plan, xp16, perm, crow_s, starts, n_atoms, glpe_pad):
    """Per-core packed input stream [128, s_cols] bf16.

    Gather blocks are slot-spread: block (tile, lane L) is [128, 128] with
    partition 32*j + f = (slot 4L+j, feat f), col = clique-in-tile, values
    pre-scaled by 1/k so the device matmul computes the segment mean.
    """
    BF16 = _bf16()
    NP = plan["np_"]
    stream = np.zeros((P, plan["s_cols"]), BF16)
    aux = {w: np.zeros((PE * w, max(plan["a_cols"][w], 1)), BF16)
           for w in (1, 2, 3)}

    xT = np.zeros((P, NP), BF16)
    realpos = np.flatnonzero(perm >= 0)
    xT[:, realpos] = xp16.T

    for grp in plan["groups"]:
        g0 = grp["off"]
        stream[:, grp["x0"]:grp["x0"] + GW] = xT[:, g0:g0 + GW]

    # gather blocks, per class (tiles of one class are contiguous)
    for k in plan["classes"]:
        if k == 0:
            continue
        q = (k + 3) // 4
        t0 = plan["class_tile0"][k]
        nk = plan["n"][k]
        idmat = perm[t0 * P:(t0 + nk) * P].reshape(nk, P)
        st = np.where(idmat >= 0, starts[idmat.clip(0)], 0)
        base = st[..., None] + np.arange(k)[None, None, :]   # [nk, P, k]
        vals = crow_s[base.clip(0, max(len(crow_s) - 1, 0))]
        vals[idmat < 0] = n_atoms
        rows = (glpe_pad[vals] * np.float32(1.0 / k)).astype(BF16)
        for i in range(nk):
            t = t0 + i
            grp = plan["groups"][t // GROUP]
            tt = t % GROUP
            X = rows[i]                                       # [P, k, 32]
            for L in range(q):
                w, c0b = grp["tlc"][(tt, L)]
                blk = X[:, 4 * L:4 * L + w, :].transpose(1, 2, 0)
                dst = stream if w == 4 else aux[w]
                dst[:, c0b:c0b + P] = blk.reshape(PE * w, P)
    return stream, aux


# --------------------------------------------------------------------------
# Bass program
# --------------------------------------------------------------------------

def _build_bass(plan, repeat=None, mode="full"):
    """mode: "full" (default) | "dma" (loads/stores only) | "compute"
    (no big DMAs; engines read a memset dummy tile).  The non-full modes
    exist only for on-hardware bottleneck attribution."""
    import concourse.bass as bass
    import concourse.bacc as bacc
    import concourse.mybir as mybir
    import concourse.tile as tile
    from concourse.masks import make_identity

    f32 = mybir.dt.float32
    bf16 = mybir.dt.bfloat16
    NP = plan["np_"]
    # consts layout: [wm: 128][tlw: 64][strip-spread lpw variants 1..4: 64
    # cols each; variant L replicates lpw on partition strips j < L]
    C_WM, C_TLW, C_SP = 0, HID, HID + 64
    c_cols = C_SP + 4 * 64

    nc = bacc.Bacc(None)
    d_stream = nc.declare_dram_parameter("stream", [P, plan["s_cols"]], bf16,
                                         isOutput=False)
    d_aux = {w: nc.declare_dram_parameter(
                 f"aux{w}", [PE * w, max(plan["a_cols"][w], 1)], bf16,
                 isOutput=False)
             for w in (1, 2, 3)}
    d_tl = nc.declare_dram_parameter("tlT", [PE, NP], bf16, isOutput=False)
    d_consts = nc.declare_dram_parameter("consts", [P, c_cols], bf16,
                                         isOutput=False)
    d_bias = nc.declare_dram_parameter("bias", [HID, 2], f32, isOutput=False)
    d_out = nc.declare_dram_parameter("outT", [P, NP], bf16, isOutput=True)

    with tile.TileContext(nc) as tc:
        with (
            tc.tile_pool(name="const", bufs=1) as cp,
            tc.tile_pool(name="st", bufs=3) as spool,
            tc.tile_pool(name="a1", bufs=3) as a1pool,
            tc.tile_pool(name="a2", bufs=3) as a2pool,
            tc.tile_pool(name="a3", bufs=3) as a3pool,
            tc.tile_pool(name="tl", bufs=3) as tlpool,
            tc.tile_pool(name="ot", bufs=3) as opool,
            tc.tile_pool(name="psF", bufs=8, space="PSUM") as psF,
        ):
            apools = {1: a1pool, 2: a2pool, 3: a3pool}
            # ---------------- constants ----------------
            cw = cp.tile([P, c_cols], bf16, tag="cw")
            nc.sync.dma_start(out=cw[:], in_=d_consts[:, :])
            bias_sb = cp.tile([HID, 2], f32, tag="bias")
            nc.sync.dma_start(out=bias_sb[:], in_=d_bias[:, :])
            if mode == "compute":
                fake_st = cp.tile([P, 2048], bf16, tag="fst")
                nc.vector.memset(fake_st[:], 0.25)
                fake_tl = cp.tile([PE, GW], bf16, tag="ftl")
                nc.vector.memset(fake_tl[:], 0.25)

            # ---------------- main loop ----------------
            import contextlib
            rep_ctx = (tc.For_i(0, repeat, 1) if repeat
                       else contextlib.nullcontext())
            rep_ctx.__enter__()

            for sg in plan["sgs"]:
                c0 = sg["c0"]
                ng = len(sg["groups"])
                if mode != "compute":
                    st = spool.tile([P, plan["max_sg_cols"]], bf16,
                                    tag="st")
                    nc.sync.dma_start(out=st[:, :sg["cols"]],
                                      in_=d_stream[:, c0:c0 + sg["cols"]])
                    # small loads go through the idle Pool queue (software
                    # DGE) so they don't serialize on the HWDGE ring behind
                    # the big stream load and the store
                    ax = {}
                    for w in (1, 2, 3):
                        nw = sg["acn"][w]
                        if plan["max_a_cols"][w] == 0:
                            continue
                        at = apools[w].tile([PE * w, plan["max_a_cols"][w]],
                                            bf16, tag=f"ax{w}")
                        ax[w] = at
                        if nw:
                            a0 = sg["ac0"][w]
                            nc.gpsimd.dma_start(out=at[:, :nw],
                                                in_=d_aux[w][:, a0:a0 + nw])
                    tl = tlpool.tile([PE, SG * GW], bf16, tag="tl")
                    nc.gpsimd.dma_start(
                        out=tl[:, :ng * GW],
                        in_=d_tl[:, sg["out0"]:sg["out0"] + ng * GW])
                ot = opool.tile([P, SG * GW], bf16, tag="ot")
                if mode == "dma":
                    nc.vector.memset(ot[:, 0:1], 0.0)

                # whole-sg phases (8 PSUM banks = 8 groups in flight):
                # all wm matmuls, then all tpe, then all lpe segment-sums,
                # then the ACT bias-add output copies.  One weight load per
                # phase run; PE streams with no cross-engine dependencies.
                glist = list(enumerate(sg["groups"])) if mode != "dma" else []
                fins = {}
                for gl, grp in glist:
                    fin = psF.tile([P, GW], f32)
                    fins[gl] = fin
                for gl, grp in glist:
                    if mode == "compute":
                        xs_ap = fake_st[:, 0:GW]
                    else:
                        xs_ap = st[:, grp["x0"] - c0:grp["x0"] - c0 + GW]
                    nc.tensor.matmul(fins[gl][:, :],
                                     lhsT=cw[:, C_WM:C_WM + HID],
                                     rhs=xs_ap, start=True, stop=False,
                                     skip_group_check=True)
                for gl, grp in glist:
                    if mode == "compute":
                        tl_ap = fake_tl[:, 0:GW]
                    else:
                        tl_ap = tl[:, gl * GW:(gl + 1) * GW]
                    nc.tensor.matmul(fins[gl][64:128, :],
                                     lhsT=cw[0:PE, C_TLW:C_TLW + 64],
                                     rhs=tl_ap,
                                     start=False, stop=grp["class0"],
                                     skip_group_check=True)
                # lpe: merged slot-spread segment matmuls (sum over edge
                # slots and project in one op, accumulating into fin[0:64]);
                # width-w segments read the [32w, *] aux stream with the
                # strip-replicated lpw sliced to w strips
                for gl, grp in glist:
                    for i, seg in enumerate(grp["segs"]):
                        w = seg["w"]
                        co = C_SP + (w - 1) * 64
                        if mode == "compute":
                            rhs = (fake_st[0:PE * w, 0:seg["ncols"]])
                        elif w == 4:
                            rhs = st[:, seg["col0"] - c0:
                                     seg["col0"] - c0 + seg["ncols"]]
                        else:
                            a0 = sg["ac0"][w]
                            rhs = ax[w][:, seg["col0"] - a0:
                                        seg["col0"] - a0 + seg["ncols"]]
                        nc.tensor.matmul(
                            fins[gl][0:64,
                                     seg["out0"]:seg["out0"] + seg["ncols"]],
                            lhsT=cw[0:PE * w, co:co + 64], rhs=rhs,
                            start=False, stop=(i == len(grp["segs"]) - 1),
                            skip_group_check=True)
                # bias-add copies PSUM -> bf16 output tile (ACT/DVE split)
                for gl, grp in glist:
                    bcol = 0 if grp["class0"] else 1
                    if gl % 2 == 0:
                        nc.scalar.add(ot[:, gl * GW:(gl + 1) * GW],
                                      fins[gl][:, :],
                                      bias_sb[:, bcol:bcol + 1])
                    else:
                        nc.vector.tensor_scalar(
                            out=ot[:, gl * GW:(gl + 1) * GW],
                            in0=fins[gl][:, :],
                            scalar1=bias_sb[:, bcol:bcol + 1], scalar2=None,
                            op0=mybir.AluOpType.add)

                # issue the store from the ACT queue: SP's in-order SEQ
                # would otherwise park on this DMA's wait and stall the
                # next super-group's input DMA dispatch (no overlap).
                nc.scalar.dma_start(
                    out=d_out[:, sg["out0"]:sg["out0"] + ng * GW],
                    in_=ot[:, :ng * GW])

            rep_ctx.__exit__(None, None, None)

    nc.compile()
    return nc


# --------------------------------------------------------------------------
# SPMD execution via PJRT (axon)
# --------------------------------------------------------------------------

def _run_spmd(nc, in_maps, bench=None):
    import jax
    import numpy as np
    from jax.sharding import Mesh, PartitionSpec
    from jax.experimental.shard_map import shard_map
    from concourse import bass2jax, mybir
    from concourse.bass2jax import _bass_exec_p, partition_id_tensor

    bass2jax.install_neuronx_cc_hook()
    n_cores = len(in_maps)
    partition_name = nc.partition_id_tensor.name if nc.partition_id_tensor else None
    in_names, out_names, out_avals, zero_outs = [], [], [], []
    for alloc in nc.m.functions[0].allocations:
        if not isinstance(alloc, mybir.MemoryLocationSet):
            continue
        name = alloc.memorylocations[0].name
        if alloc.kind == "ExternalInput":
            if name != partition_name:
                in_names.append(name)
        elif alloc.kind == "ExternalOutput":
            out_names.append(name)
            shape = tuple(alloc.tensor_shape)
            dtype = mybir.dt.np(alloc.dtype)
            out_avals.append(jax.core.ShapedArray(shape, dtype))
            zero_outs.append(np.zeros(shape, dtype))
    n_params = len(in_names)
    n_outs = len(out_avals)
    in_names.extend(out_names)
    if partition_name is not None:
        in_names.append(partition_name)

    def _body(*args):
        operands = list(args)
        if partition_name is not None:
            operands.append(partition_id_tensor())
        return tuple(_bass_exec_p.bind(
            *operands, out_avals=tuple(out_avals), in_names=tuple(in_names),
            out_names=tuple(out_names), lowering_input_output_aliases=(),
            sim_require_finite=True, sim_require_nnan=True, nc=nc))

    devices = jax.devices()[:n_cores]
    mesh = Mesh(np.asarray(devices), ("core",))
    in_specs = (PartitionSpec("core"),) * (n_params + n_outs)
    out_specs = (PartitionSpec("core"),) * len(out_names)
    sharded = jax.jit(shard_map(_body, mesh=mesh, in_specs=in_specs,
                                out_specs=out_specs, check_rep=False),
                      keep_unused=True)
    concat_in = [np.concatenate([np.asarray(m[in_names[i]]) for m in in_maps], axis=0)
                 for i in range(n_params)]
    concat_zeros = [np.zeros((n_cores * z.shape[0], *z.shape[1:]), z.dtype)
                    for z in zero_outs]
    sharding = jax.sharding.NamedSharding(mesh, PartitionSpec("core"))
    dev_in = [jax.device_put(a, sharding) for a in concat_in + concat_zeros]
    out_arrs = jax.block_until_ready(sharded(*dev_in))

    if bench is not None:
        import time
        iters = int(bench.get("iters", 10))
        times = []
        for _ in range(iters):
            t0 = time.perf_counter()
            jax.block_until_ready(sharded(*dev_in))
            times.append(time.perf_counter() - t0)
        bench["times"] = times
        bench["min_wall_ns"] = int(min(times) * 1e9)

    return [{name: np.asarray(out_arrs[i]).reshape(n_cores, *out_avals[i].shape)[c]
             for i, name in enumerate(out_names)} for c in range(n_cores)]


# --------------------------------------------------------------------------
# entry point
# --------------------------------------------------------------------------

def kernel(x_clique, tree_lpe, graph_lpe, tree_degree, row, col,
           deg_emb, deg_lin_w, deg_lin_b, deg_merge_w, deg_merge_b,
           tree_lpe_w, tree_lpe_b, lpe_w, lpe_b, _bench=None):
    BF16 = _bf16()

    x_clique = np.asarray(x_clique, np.float32)
    tree_lpe = np.asarray(tree_lpe, np.float32)
    graph_lpe = np.asarray(graph_lpe, np.float32)
    tree_degree = np.asarray(tree_degree).astype(np.int64)
    row = np.asarray(row).astype(np.int64)
    col = np.asarray(col).astype(np.int64)
    deg_emb = np.asarray(deg_emb, np.float32)
    deg_lin_w = np.asarray(deg_lin_w, np.float32)
    deg_lin_b = np.asarray(deg_lin_b, np.float32)
    deg_merge_w = np.asarray(deg_merge_w, np.float32)
    deg_merge_b = np.asarray(deg_merge_b, np.float32)
    tree_lpe_w = np.asarray(tree_lpe_w, np.float32)
    tree_lpe_b = np.asarray(tree_lpe_b, np.float32)
    lpe_w = np.asarray(lpe_w, np.float32)
    lpe_b = np.asarray(lpe_b, np.float32)

    n_clique = x_clique.shape[0]
    n_atoms = graph_lpe.shape[0]
    assert n_clique % N_CORES == 0
    cpc = n_clique // N_CORES

    # degree table folded on host: T = relu(deg_emb @ W1 + b1)
    degfeat = np.maximum(deg_emb @ deg_lin_w + deg_lin_b, 0.0)

    # ---- host index prep: partition edges by owning core, count per clique
    order = np.argsort(col, kind="stable")
    col_s = col[order]
    row_s = row[order]
    bounds = np.searchsorted(col_s, np.arange(N_CORES + 1) * cpc)

    cnts, crows = [], []
    for c in range(N_CORES):
        lo, hi = bounds[c], bounds[c + 1]
        cc = col_s[lo:hi] - c * cpc
        cnts.append(np.bincount(cc, minlength=cpc).astype(np.int64))
        crows.append(row_s[lo:hi])

    kmax = int(max(int(c.max(initial=0)) for c in cnts))
    plan = _plan(cnts, kmax)

    glpe_pad = np.vstack([np.nan_to_num(graph_lpe, nan=0.0),
                          np.zeros((1, PE), np.float32)])

    # consts: [wm 128][tlw 64][strip-spread lpw variants 1..4]  (bf16)
    c_cols = HID + 64 + 4 * 64
    consts = np.zeros((P, c_cols), BF16)
    consts[:, :HID] = deg_merge_w.astype(BF16)
    consts[0:PE, HID:HID + 64] = tree_lpe_w.astype(BF16)
    lpw16 = lpe_w.astype(BF16)
    for L in range(1, 5):
        co = HID + 64 + (L - 1) * 64
        for j in range(L):
            consts[PE * j:PE * (j + 1), co:co + 64] = lpw16

    bias = np.zeros((HID, 2), np.float32)
    bias[:, 0] = deg_merge_b + np.concatenate([np.zeros(64, np.float32),
                                               tree_lpe_b])
    bias[:, 1] = bias[:, 0] + np.concatenate([lpe_b, np.zeros(64, np.float32)])

    in_maps = []
    unshard = []
    for c in range(N_CORES):
        cnt = cnts[c]
        perm, realpos, realids = _perm_arrays(plan, cnt)
        crow_s = crows[c]
        starts = np.zeros(cpc, np.int64)
        cs = np.cumsum(cnt)
        starts[1:] = cs[:-1]

        x_c = x_clique[c * cpc:(c + 1) * cpc]
        tl_c = tree_lpe[c * cpc:(c + 1) * cpc]
        deg_c = tree_degree[c * cpc:(c + 1) * cpc]

        xp16 = (x_c[realids] + degfeat[deg_c[realids]]).astype(BF16)
        tlT = np.zeros((PE, plan["np_"]), BF16)
        tlT[:, realpos] = np.nan_to_num(tl_c[realids], nan=0.0).astype(BF16).T

        stream, aux = _core_stream(plan, xp16, perm, crow_s, starts, n_atoms,
                                   glpe_pad)
        in_maps.append(dict(stream=stream, aux1=aux[1], aux2=aux[2],
                            aux3=aux[3], tlT=tlT, consts=consts, bias=bias))
        unshard.append((realpos, realids))

    cache_key = (tuple(plan["tiles"]),)
    nc = _COMPILE_CACHE.get(cache_key)
    if nc is None:
        nc = _build_bass(plan)
        _COMPILE_CACHE[cache_key] = nc

    results = _run_spmd(nc, in_maps, bench=_bench)

    # true HW time: run repeat-R variants of the program (device-side loop);
    # the wall-time slope vs R is pure device time, dispatch cancels out.
    if _bench is not None and _bench.get("hw_probe"):
        walls = {}
        for R in _bench["hw_probe"]:
            ncR = _build_bass(plan, repeat=R)
            b2 = {"iters": _bench.get("iters", 8)}
            _run_spmd(ncR, in_maps, bench=b2)
            walls[R] = min(b2["times"])
        rs = sorted(walls)
        _bench["walls"] = walls
        _bench["hw_ns_est"] = int(
            (walls[rs[-1]] - walls[rs[0]]) / (rs[-1] - rs[0]) * 1e9)

    out = np.empty((n_clique, HID), np.float32)
    for c in range(N_CORES):
        realpos, realids = unshard[c]
        outT = results[c]["outT"]  # [128, NP] bf16
        out[c * cpc + realids] = outT.T[realpos].astype(np.float32)
    return out
